# revision 21
# baseline (speedup 1.0000x reference)
"""Trainium2 Bass kernel for GroupedQueryAttention with 1-bit quantized linears.

Sharding: 8 cores = 2 batches x 4 token-interleaved groups.
Core c handles batch b=c//4 and tokens t with t%4 == i (i=c%4), i.e. 512
query tokens per core.  Every core computes full K/V for its batch
(replicated), all 16 heads for its own 512 queries, and the full O
projection for those rows.  Host gathers by re-interleaving rows.

v2 restructure vs baseline:
 - no DRAM bf16 round-trip for x: f32 tiles load straight to SBUF,
   engines cast to bf16, SBUF->SBUF DMA transpose into XT.
 - weight quantization reads the f32 staging tile directly (reduce DVE,
   sign ACT, scale-mult gpsimd/DVE).
 - exp over PAIRED score tiles ([128,1024] PSUM spans 2 banks) to halve
   ACT per-op overhead.
 - softmax reciprocal broadcast via K=1 matmul instead of DRAM trip.
 - o-weight quant overlaps K/V projections; owT transpose overlaps
   attention.

Program is identical across cores; all per-core variation is input data.
"""

import sys

sys.path.insert(0, "/opt/trn_rl_repo")

import numpy as np
import ml_dtypes

import concourse.bacc as bacc
import concourse.bass as bass
import concourse.mybir as mybir
import concourse.tile as tile

F32 = mybir.dt.float32
F16 = mybir.dt.float16
BF16 = mybir.dt.bfloat16

B, T, D = 2, 2048, 2048
H, HK, HD = 16, 4, 128
G = 128
THETA = 1000000.0
NC = 8
TQ = T // 4          # 512 query tokens per core
QT = TQ // 128       # 4 query tiles
DT = D // 128        # 16 din tiles
NKT = T // 128       # 16 key tiles
NPAIR = NKT // 2     # 8 key-tile pairs

ALPHA_K = 1.0 / G
ALPHA_Q = (HD ** -0.5) / G


def _bcast(ap_small, like_ap):
    a, b = bass.broadcast_tensor_aps(like_ap, ap_small)
    return b


def build_program():
    nc = bacc.Bacc("TRN2", target_bir_lowering=False, debug=False, num_devices=NC)

    x = nc.dram_tensor("x", [T, D], F32, kind="ExternalInput").ap()
    xq = nc.dram_tensor("xq", [TQ, D], F32, kind="ExternalInput").ap()
    qw = nc.dram_tensor("qw", [H * HD, D], F32, kind="ExternalInput").ap()
    kw = nc.dram_tensor("kw", [HK * HD, D], F32, kind="ExternalInput").ap()
    vw = nc.dram_tensor("vw", [HK * HD, D], F32, kind="ExternalInput").ap()
    ow = nc.dram_tensor("ow", [D, H * HD], F32, kind="ExternalInput").ap()
    cosk = nc.dram_tensor("cosk", [HD, T], F16, kind="ExternalInput").ap()
    sinkr = nc.dram_tensor("sinkr", [HD, T], F16, kind="ExternalInput").ap()
    cosq = nc.dram_tensor("cosq", [HD, TQ], F16, kind="ExternalInput").ap()
    sinqr = nc.dram_tensor("sinqr", [HD, TQ], F16, kind="ExternalInput").ap()
    dmask = nc.dram_tensor("dmask", [128, 32], BF16, kind="ExternalInput").ap()
    out = nc.dram_tensor("out", [TQ, D], F32, kind="ExternalOutput").ap()

    with tile.TileContext(nc) as tc:
        build_tile_kernel(nc, tc, x, xq, qw, kw, vw, ow, cosk, sinkr, cosq,
                          sinqr, dmask, out)
    nc.compile()
    return nc


def build_tile_kernel(nc, tc, x, xq, qw, kw, vw, ow, cosk, sinkr, cosq, sinqr,
                      dmask, out):
    from contextlib import ExitStack

    ctx = ExitStack()
    with ctx:
        # ------- long-lived pools --------
        dram = ctx.enter_context(tc.tile_pool(name="dram", bufs=1, space="DRAM"))
        const = ctx.enter_context(tc.tile_pool(name="const", bufs=1))
        resid = ctx.enter_context(tc.tile_pool(name="resid", bufs=1))

        wqd = dram.tile([D, H * HD], BF16)      # quantized o-weights (row major)

        dmask_sb = const.tile([128, 32], BF16)
        nc.sync.dma_start(dmask_sb, dmask)
        # [128,128] of G*G: sum-matmul output lands broadcast on all 128
        # partitions, so the reciprocal runs on 128 lanes (no [1,512] crawl)
        ones16k = const.tile([128, 128], BF16)
        nc.gpsimd.memset(ones16k, float(G * G))

        cosk_sb = const.tile([128, T], F16)
        sinkr_sb = const.tile([128, T], F16)
        cosq_sb = const.tile([128, TQ], F16)
        sinqr_sb = const.tile([128, TQ], F16)
        nc.sync.dma_start(cosk_sb, cosk)
        nc.sync.dma_start(sinkr_sb, sinkr)
        nc.sync.dma_start(cosq_sb, cosq)
        nc.sync.dma_start(sinqr_sb, sinqr)

        # residents alive through attention (48 KB/partition)
        QT_t = resid.tile([128, H, TQ], BF16)    # roped q^T  [d, h, t]
        KT_t = resid.tile([128, HK, T], BF16)    # roped k^T  [d, hk, t]
        V_t = resid.tile([128, NKT, HK * HD], BF16)  # v [t, kv-dim]

        # ============== phase 1: staging + projections ==================
        with tc.tile_pool(name="xtp", bufs=1) as pxt, \
             tc.tile_pool(name="xstage", bufs=2) as xs, \
             tc.tile_pool(name="wst", bufs=2) as wst_p, \
             tc.tile_pool(name="wqt", bufs=2) as wqt_p, \
             tc.tile_pool(name="ssum", bufs=2) as ssum_p, \
             tc.tile_pool(name="rtmp", bufs=1) as rtmp, \
             tc.tile_pool(name="proj_ps", bufs=4, space="PSUM") as pps:

            XT = pxt.tile([128, DT, T], BF16)     # x^T (din-major), 64 KB/p

            def stage_x(src_ap, dst_T, rows, idx):
                """Load f32 rows, cast to bf16, DMA-transpose into dst_T."""
                xb = xs.tile([128, D], BF16, tag="xb", bufs=2)
                for half in range(2):
                    xf = xs.tile([128, D // 2], F32, tag="xf", bufs=3)
                    nc.scalar.dma_start(
                        xf, src_ap[rows * 128:(rows + 1) * 128,
                                   half * (D // 2):(half + 1) * (D // 2)])
                    if (2 * idx + half) % 2 == 0:
                        nc.scalar.copy(
                            xb[:, half * (D // 2):(half + 1) * (D // 2)], xf)
                    else:
                        nc.vector.tensor_copy(
                            xb[:, half * (D // 2):(half + 1) * (D // 2)], xf)
                nc.sync.dma_start_transpose(dst_T, xb[:])

            def quant_tile(w_ap, row_tile, out_T=None, out_rowmajor=None,
                           mult_eng=None):
                """Load 128 rows of f32 w, 1-bit quantize -> bf16 (x G scale).

                Processed in two 1024-col halves to bound SBUF staging; sign
                is written into wqt, then scaled in place.
                """
                HD2 = D // 2
                for hf in range(2):
                    wst = wst_p.tile([128, HD2], F32, tag="wst", bufs=3)
                    nc.sync.dma_start(
                        wst, w_ap[row_tile * 128:(row_tile + 1) * 128,
                                  hf * HD2:(hf + 1) * HD2])
                    ssum = ssum_p.tile([128, DT // 2], F32, tag="ssum")
                    nc.vector.tensor_reduce(
                        ssum, wst.rearrange("p (g c) -> p g c", c=G),
                        axis=mybir.AxisListType.X, op=mybir.AluOpType.add,
                        apply_absolute_value=True)
                    wqt = wqt_p.tile([128, HD2], BF16, tag="wqt", bufs=3)
                    nc.scalar.sign(wqt, wst)
                    sv = ssum.rearrange("p (g o) -> p g o", o=1)
                    gv = wqt.rearrange("p (g c) -> p g c", c=G)
                    (mult_eng or nc.vector).tensor_tensor(
                        gv, gv, _bcast(sv, gv), op=mybir.AluOpType.mult)
                    if out_T is not None:
                        nc.sync.dma_start_transpose(
                            out_T[:, hf * (DT // 2):(hf + 1) * (DT // 2), :],
                            wqt[:])
                    if out_rowmajor is not None:
                        nc.sync.dma_start(
                            out_rowmajor[:, hf * HD2:(hf + 1) * HD2], wqt[:])

            def rope_evac(ps, cos_sb, sinr_sb, col0, width, out_ap):
                """out = ps*cos + rot(ps)*sinr  (cast bf16)."""
                t1 = rtmp.tile([128, width], F32, tag="t1", bufs=1)
                t2 = rtmp.tile([128, width], F32, tag="t2", bufs=1)
                cs = cos_sb[:, col0:col0 + width]
                sr = sinr_sb[:, col0:col0 + width]
                nc.vector.tensor_tensor(t1, ps, cs, op=mybir.AluOpType.mult)
                nc.vector.tensor_tensor(t2[0:64, :], ps[64:128, :], sr[0:64, :],
                                        op=mybir.AluOpType.mult)
                nc.vector.tensor_tensor(t2[64:128, :], ps[0:64, :],
                                        sr[64:128, :], op=mybir.AluOpType.mult)
                nc.gpsimd.tensor_tensor(out_ap, t1, t2, op=mybir.AluOpType.add)

            # --- interleaved: xq staging, Q heads, x staging, K chunks ---
            # Q head h projects while x tile h stages; after head 4c+3 the
            # K chunk c (x tiles 4c..4c+3) projects.  PE never waits on DMA.
            with tc.tile_pool(name="qk", bufs=1) as qk_p:
                XTq = qk_p.tile([128, DT, TQ], BF16, tag="XTq", bufs=1)
                for ti in range(QT):
                    stage_x(xq, XTq[:, :, ti * 128:(ti + 1) * 128], ti, ti)

                kwT = []
                for hk in range(HK):
                    wT = qk_p.tile([128, DT, 128], BF16, tag="kwT", bufs=4)
                    quant_tile(kw, hk, out_T=wT[:])
                    kwT.append(wT)

                for h in range(H):
                    stage_x(x, XT[:, :, h * 128:(h + 1) * 128], h, h + QT)
                    wT = qk_p.tile([128, DT, 128], BF16, tag="qwT", bufs=2)
                    quant_tile(qw, h, out_T=wT[:], mult_eng=nc.gpsimd)
                    ps = pps.tile([128, TQ], F32, tag="ps")
                    for dt in range(DT):
                        nc.tensor.matmul(ps, wT[:, dt, :], XTq[:, dt, :],
                                         start=(dt == 0), stop=(dt == DT - 1))
                    rope_evac(ps, cosq_sb, sinqr_sb, 0, TQ, QT_t[:, h, :])

                    if h % 4 == 3:
                        tc4 = h // 4
                        for hk in range(HK):
                            ps = pps.tile([128, 512], F32, tag="ps")
                            for dt in range(DT):
                                nc.tensor.matmul(
                                    ps, kwT[hk][:, dt, :],
                                    XT[:, dt, tc4 * 512:(tc4 + 1) * 512],
                                    start=(dt == 0), stop=(dt == DT - 1))
                            rope_evac(ps, cosk_sb, sinkr_sb, tc4 * 512, 512,
                                      KT_t[:, hk, tc4 * 512:(tc4 + 1) * 512])

            # --- V projection + o-weight quant ---
            with tc.tile_pool(name="vq", bufs=1) as vq_p:
                vq = vq_p.tile([128, DT, HK * HD], BF16)
                for rv in range(HK * HD // 128):
                    quant_tile(vw, rv, out_T=vq[:, :, rv * 128:(rv + 1) * 128])
                for tch in range(NKT):
                    ps = pps.tile([128, HK * HD], F32, tag="ps")
                    for dt in range(DT):
                        nc.tensor.matmul(ps,
                                         XT[:, dt, tch * 128:(tch + 1) * 128],
                                         vq[:, dt, :],
                                         start=(dt == 0), stop=(dt == DT - 1))
                    nc.scalar.copy(V_t[:, tch, :], ps)
                    # o-weight quant engine work rides along with V proj
                    if tch % 2 == 0:
                        ro = tch // 2
                        quant_tile(ow, ro,
                                   out_rowmajor=wqd[ro * 128:(ro + 1) * 128, :],
                                   mult_eng=nc.gpsimd)
                for ro in range(8, D // 128):
                    quant_tile(ow, ro,
                               out_rowmajor=wqd[ro * 128:(ro + 1) * 128, :],
                               mult_eng=nc.gpsimd)

        # ============== phase 2: attention + output projection ==========
        with tc.tile_pool(name="att_res", bufs=1) as ares:

            OT_t = ares.tile([128, H, TQ], BF16)   # attn out^T [dv, h, q]
            owT = ares.tile([128, H, D], BF16)     # o-weights^T [dH, ht, dout]

            with tc.tile_pool(name="attn", bufs=2) as apool, \
                 tc.tile_pool(name="st_ps", bufs=2, space="PSUM") as stp, \
                 tc.tile_pool(name="sum_ps", bufs=1, space="PSUM") as sump, \
                 tc.tile_pool(name="o_ps", bufs=2, space="PSUM") as op:
                attention_heads(nc, tc, apool, stp, sump, op, KT_t, QT_t, V_t,
                                OT_t, owT, wqd, dmask_sb, ones16k)

            # --- output projection ---
            with tc.tile_pool(name="oproj", bufs=2) as opool, \
                 tc.tile_pool(name="op_ps", bufs=4, space="PSUM") as opp:
                for m in range(QT):
                    osb = opool.tile([128, D], F32, tag="osb")
                    for oc in range(4):
                        ps = opp.tile([128, 512], F32, tag="ps")
                        for ht in range(H):
                            nc.tensor.matmul(ps,
                                             OT_t[:, ht, m * 128:(m + 1) * 128],
                                             owT[:, ht, oc * 512:(oc + 1) * 512],
                                             start=(ht == 0), stop=(ht == H - 1))
                        if oc % 2 == 0:
                            nc.vector.tensor_copy(
                                osb[:, oc * 512:(oc + 1) * 512], ps)
                        else:
                            nc.scalar.copy(osb[:, oc * 512:(oc + 1) * 512], ps)
                    nc.sync.dma_start(out[m * 128:(m + 1) * 128, :], osb)


def attention_heads(nc, tc, apool, stp, sump, op, KT_t, QT_t, V_t, OT_t, owT,
                    wqd, dmask_sb, ones16k):
            for h in range(H):
                hk = h // 4
                ps_o = op.tile([128, TQ], F32, tag="ps_o")
                ps_sum = sump.tile([128, TQ], F32, tag="ps_sum")
                for p in range(NPAIR):
                    kt0, kt1 = 2 * p, 2 * p + 1
                    q0, q1 = 32 * kt0, 32 * kt1
                    ps_st = stp.tile([128, 2 * TQ], F32, tag="ps_st")
                    nc.tensor.matmul(ps_st[:, q0:TQ],
                                     KT_t[:, hk, kt0 * 128:(kt0 + 1) * 128],
                                     QT_t[:, h, q0:], start=True, stop=True)
                    nc.tensor.matmul(ps_st[:, TQ + q1:2 * TQ],
                                     KT_t[:, hk, kt1 * 128:(kt1 + 1) * 128],
                                     QT_t[:, h, q1:], start=True, stop=True)
                    pt = apool.tile([128, 2 * TQ], BF16, tag="pt", bufs=4)
                    # one exp op over both halves, strided to skip the gap
                    nc.scalar.activation(
                        pt.rearrange("p (k q) -> p k q", k=2)[:, :, q0:],
                        ps_st.rearrange("p (k q) -> p k q", k=2)[:, :, q0:],
                        mybir.ActivationFunctionType.Exp)
                    # diagonal strip masks (multiplicative 0/1)
                    nc.gpsimd.tensor_tensor(pt[:, q0:q0 + 32], pt[:, q0:q0 + 32],
                                            dmask_sb, op=mybir.AluOpType.mult)
                    nc.gpsimd.tensor_tensor(pt[:, TQ + q1:TQ + q1 + 32],
                                            pt[:, TQ + q1:TQ + q1 + 32],
                                            dmask_sb, op=mybir.AluOpType.mult)
                    # denominator + attn@V accumulation
                    nc.tensor.matmul(ps_sum[:, q0:], ones16k, pt[:, q0:TQ],
                                     start=(p == 0), stop=False)
                    nc.tensor.matmul(ps_sum[:, q1:], ones16k,
                                     pt[:, TQ + q1:2 * TQ],
                                     start=False, stop=(p == NPAIR - 1))
                    nc.tensor.matmul(ps_o[:, q0:],
                                     V_t[:, kt0, hk * HD:(hk + 1) * HD],
                                     pt[:, q0:TQ], start=(p == 0), stop=False)
                    nc.tensor.matmul(ps_o[:, q1:],
                                     V_t[:, kt1, hk * HD:(hk + 1) * HD],
                                     pt[:, TQ + q1:2 * TQ],
                                     start=False, stop=(p == NPAIR - 1))
                # sums arrive broadcast on all 128 partitions: wide reciprocal
                RQb = apool.tile([128, TQ], F32, tag="RQb", bufs=2)
                nc.vector.reciprocal(RQb, ps_sum)
                nc.vector.tensor_tensor(OT_t[:, h, :], ps_o, RQb,
                                        op=mybir.AluOpType.mult)
                # interleave owT transposes with attention
                if h >= 8:
                    for ht in (2 * (h - 8), 2 * (h - 8) + 1):
                        nc.sync.dma_start_transpose(
                            owT[:, ht, :], wqd[:, ht * 128:(ht + 1) * 128])


# ---------------------------------------------------------------------------
# host side
# ---------------------------------------------------------------------------
_CACHE = {}


def _tables():
    inv = 1.0 / (THETA ** (np.arange(0, HD, 2, dtype=np.float64) / HD))
    t = np.arange(T, dtype=np.float64)
    fr = np.outer(t, inv)                      # [T, 64]
    emb = np.concatenate([fr, fr], axis=1)     # [T, 128]
    cosT = np.cos(emb).T                       # [128, T] float64
    sinT = np.sin(emb).T
    sinr = np.empty_like(sinT)
    sinr[0:64] = -sinT[0:64]
    sinr[64:128] = sinT[64:128]
    return cosT, sinT, sinr


def make_in_maps(hidden, q_w, k_w, v_w, o_w):
    cosT, sinT, sinr = _tables()
    f16 = np.float16
    in_maps = []
    for c in range(NC):
        b, i = c // 4, c % 4
        xb_ = np.ascontiguousarray(hidden[b])
        xq_ = np.ascontiguousarray(hidden[b][i::4, :])
        cq = np.ascontiguousarray(cosT[:, i::4] * ALPHA_Q).astype(f16)
        sq = np.ascontiguousarray(sinr[:, i::4] * ALPHA_Q).astype(f16)
        # dmask[r, c] = 1 iff key-local r <= 4c + i (diagonal 128x32 strip)
        r = np.arange(128)[:, None]
        cc = np.arange(32)[None, :]
        dm = (r <= 4 * cc + i).astype(ml_dtypes.bfloat16)
        in_maps.append({
            "x": xb_, "xq": xq_, "qw": q_w, "kw": k_w, "vw": v_w, "ow": o_w,
            "cosk": np.ascontiguousarray(cosT * ALPHA_K).astype(f16),
            "sinkr": np.ascontiguousarray(sinr * ALPHA_K).astype(f16),
            "cosq": cq, "sinqr": sq, "dmask": dm,
        })
    return in_maps


def kernel(hidden, q_w, k_w, v_w, o_w):
    hidden = np.asarray(hidden, dtype=np.float32)
    q_w = np.ascontiguousarray(np.asarray(q_w, dtype=np.float32))
    k_w = np.ascontiguousarray(np.asarray(k_w, dtype=np.float32))
    v_w = np.ascontiguousarray(np.asarray(v_w, dtype=np.float32))
    o_w = np.ascontiguousarray(np.asarray(o_w, dtype=np.float32))

    if "nc" not in _CACHE:
        _CACHE["nc"] = build_program()
    nc = _CACHE["nc"]

    in_maps = make_in_maps(hidden, q_w, k_w, v_w, o_w)
    from concourse.bass_utils import run_bass_kernel_spmd
    res = run_bass_kernel_spmd(nc, in_maps, core_ids=list(range(NC)))
    out = np.empty((B, T, D), dtype=np.float32)
    for c in range(NC):
        b, i = c // 4, c % 4
        out[b, i::4, :] = res.results[c]["out"]
    return out


if __name__ == "__main__":
    print("building program...")
    nc = build_program()
    print("BUILD OK")


# revision 29
# speedup vs baseline: 1.1165x; 1.1165x over previous
"""Trainium2 Bass kernel for GroupedQueryAttention with 1-bit quantized linears.

Sharding: 8 cores = 2 batches x 4 token-interleaved groups.
Core c handles batch b=c//4 and tokens t with t%4 == i (i=c%4), i.e. 512
query tokens per core.  Every core computes full K/V for its batch
(replicated), all 16 heads for its own 512 queries, and the full O
projection for those rows.  Host gathers by re-interleaving rows.

v2 restructure vs baseline:
 - no DRAM bf16 round-trip for x: f32 tiles load straight to SBUF,
   engines cast to bf16, SBUF->SBUF DMA transpose into XT.
 - weight quantization reads the f32 staging tile directly (reduce DVE,
   sign ACT, scale-mult gpsimd/DVE).
 - exp over PAIRED score tiles ([128,1024] PSUM spans 2 banks) to halve
   ACT per-op overhead.
 - softmax reciprocal broadcast via K=1 matmul instead of DRAM trip.
 - o-weight quant overlaps K/V projections; owT transpose overlaps
   attention.

Program is identical across cores; all per-core variation is input data.
"""

import sys

sys.path.insert(0, "/opt/trn_rl_repo")

import numpy as np
import ml_dtypes

import concourse.bacc as bacc
import concourse.bass as bass
import concourse.mybir as mybir
import concourse.tile as tile

F32 = mybir.dt.float32
F16 = mybir.dt.float16
BF16 = mybir.dt.bfloat16

B, T, D = 2, 2048, 2048
H, HK, HD = 16, 4, 128
G = 128
THETA = 1000000.0
NC = 8
TQ = T // 4          # 512 query tokens per core
QT = TQ // 128       # 4 query tiles
DT = D // 128        # 16 din tiles
NKT = T // 128       # 16 key tiles
NPAIR = NKT // 2     # 8 key-tile pairs

ALPHA_K = 1.0 / G
ALPHA_Q = (HD ** -0.5) / G


def _bcast(ap_small, like_ap):
    a, b = bass.broadcast_tensor_aps(like_ap, ap_small)
    return b


def build_program():
    nc = bacc.Bacc("TRN2", target_bir_lowering=False, debug=False, num_devices=NC)

    x = nc.dram_tensor("x", [T, D], F32, kind="ExternalInput").ap()
    xq = nc.dram_tensor("xq", [TQ, D], F32, kind="ExternalInput").ap()
    qw = nc.dram_tensor("qw", [H * HD, D], F32, kind="ExternalInput").ap()
    kw = nc.dram_tensor("kw", [HK * HD, D], F32, kind="ExternalInput").ap()
    vw = nc.dram_tensor("vw", [HK * HD, D], F32, kind="ExternalInput").ap()
    ow = nc.dram_tensor("ow", [D, H * HD], F32, kind="ExternalInput").ap()
    cosk = nc.dram_tensor("cosk", [HD, T], F16, kind="ExternalInput").ap()
    sinkr = nc.dram_tensor("sinkr", [HD, T], F16, kind="ExternalInput").ap()
    cosq = nc.dram_tensor("cosq", [HD, TQ], F16, kind="ExternalInput").ap()
    sinqr = nc.dram_tensor("sinqr", [HD, TQ], F16, kind="ExternalInput").ap()
    dmask = nc.dram_tensor("dmask", [128, 32], BF16, kind="ExternalInput").ap()
    out = nc.dram_tensor("out", [TQ, D], F32, kind="ExternalOutput").ap()

    with tile.TileContext(nc) as tc:
        build_tile_kernel(nc, tc, x, xq, qw, kw, vw, ow, cosk, sinkr, cosq,
                          sinqr, dmask, out)
    nc.compile()
    return nc


def build_tile_kernel(nc, tc, x, xq, qw, kw, vw, ow, cosk, sinkr, cosq, sinqr,
                      dmask, out):
    from contextlib import ExitStack

    ctx = ExitStack()
    with ctx:
        # ------- long-lived pools --------
        dram = ctx.enter_context(tc.tile_pool(name="dram", bufs=1, space="DRAM"))
        const = ctx.enter_context(tc.tile_pool(name="const", bufs=1))
        resid = ctx.enter_context(tc.tile_pool(name="resid", bufs=1))

        wqd = dram.tile([D, H * HD], BF16)      # quantized o-weights (row major)

        dmask_sb = const.tile([128, 32], BF16)
        nc.sync.dma_start(dmask_sb, dmask)
        # [128,128] of G*G: sum-matmul output lands broadcast on all 128
        # partitions, so the reciprocal runs on 128 lanes (no [1,512] crawl)
        ones16k = const.tile([128, 128], BF16)
        nc.gpsimd.memset(ones16k, float(G * G))

        cosk_sb = const.tile([128, T], F16)
        sinkr_sb = const.tile([128, T], F16)
        cosq_sb = const.tile([128, TQ], F16)
        sinqr_sb = const.tile([128, TQ], F16)
        nc.sync.dma_start(cosk_sb, cosk)
        nc.sync.dma_start(sinkr_sb, sinkr)
        nc.sync.dma_start(cosq_sb, cosq)
        nc.sync.dma_start(sinqr_sb, sinqr)

        # residents alive through attention (48 KB/partition)
        QT_t = resid.tile([128, H, TQ], BF16)    # roped q^T  [d, h, t]
        KT_t = resid.tile([128, HK, T], BF16)    # roped k^T  [d, hk, t]
        V_t = resid.tile([128, NKT, HK * HD], BF16)  # v [t, kv-dim]

        # ============== phase 1: staging + projections ==================
        with tc.tile_pool(name="xtp", bufs=1) as pxt, \
             tc.tile_pool(name="xstage", bufs=2) as xs, \
             tc.tile_pool(name="wst", bufs=2) as wst_p, \
             tc.tile_pool(name="wqt", bufs=2) as wqt_p, \
             tc.tile_pool(name="ssum", bufs=2) as ssum_p, \
             tc.tile_pool(name="rtmp", bufs=1) as rtmp, \
             tc.tile_pool(name="proj_ps", bufs=4, space="PSUM") as pps:

            XT = pxt.tile([128, DT, T], BF16)     # x^T (din-major), 64 KB/p

            def stage_x(src_ap, dst_T, rows, idx):
                """Load f32 rows, cast to bf16, DMA-transpose into dst_T."""
                xb = xs.tile([128, D], BF16, tag="xb", bufs=2)
                for half in range(2):
                    xf = xs.tile([128, D // 2], F32, tag="xf", bufs=3)
                    nc.scalar.dma_start(
                        xf, src_ap[rows * 128:(rows + 1) * 128,
                                   half * (D // 2):(half + 1) * (D // 2)])
                    nc.scalar.copy(
                        xb[:, half * (D // 2):(half + 1) * (D // 2)], xf)
                nc.sync.dma_start_transpose(dst_T, xb[:])

            U16 = mybir.dt.uint16
            U32 = mybir.dt.uint32

            def quant_tile(w_ap, row_tile, out_T=None, out_rowmajor=None,
                           or_eng=None):
                """Load 128 rows of w as bf16, 1-bit quantize (x G scale).

                Copysign in u32 bit-space over bf16 pairs:
                wq = (w & 0x80008000) | ((scale<<16)|scale).
                Single-engine (DVE) chain after the gpsimd cast-load.
                """
                wst = wst_p.tile([128, D], BF16, tag="wst")
                nc.gpsimd.dma_start(
                    wst, w_ap[row_tile * 128:(row_tile + 1) * 128, :])
                ssum = ssum_p.tile([128, DT], F32, tag="ssum")
                nc.vector.tensor_reduce(
                    ssum, wst.rearrange("p (g c) -> p g c", c=G),
                    axis=mybir.AxisListType.X, op=mybir.AluOpType.add,
                    apply_absolute_value=True)
                ssb = ssum_p.tile([128, DT], BF16, tag="ssb")
                nc.vector.tensor_copy(ssb, ssum)
                s32 = ssum_p.tile([128, DT], U32, tag="s32")
                nc.vector.tensor_copy(s32, ssb.bitcast(U16))     # zero-extend
                spair = ssum_p.tile([128, DT], U32, tag="spair")
                nc.vector.tensor_scalar(
                    spair, s32, 16, None,
                    op0=mybir.AluOpType.logical_shift_left)
                nc.vector.tensor_tensor(spair, spair, s32,
                                        op=mybir.AluOpType.bitwise_or)
                wqt = wqt_p.tile([128, D], BF16, tag="wqt")
                nc.vector.tensor_scalar(
                    wqt.bitcast(U32), wst.bitcast(U32), 0x80008000, None,
                    op0=mybir.AluOpType.bitwise_and)
                sv = spair.rearrange("p (g o) -> p g o", o=1)
                gv = wqt.bitcast(U32).rearrange("p (g c) -> p g c", c=G // 2)
                nc.vector.tensor_tensor(
                    gv, gv, _bcast(sv, gv), op=mybir.AluOpType.bitwise_or)
                if out_T is not None:
                    nc.sync.dma_start_transpose(out_T, wqt[:])
                if out_rowmajor is not None:
                    nc.sync.dma_start(out_rowmajor, wqt[:])

            def rope_evac(ps, cos_sb, sinr_sb, col0, width, out_ap):
                """out = ps*cos + rot(ps)*sinr  (cast bf16)."""
                t1 = rtmp.tile([128, width], F32, tag="t1", bufs=1)
                t2 = rtmp.tile([128, width], F32, tag="t2", bufs=1)
                cs = cos_sb[:, col0:col0 + width]
                sr = sinr_sb[:, col0:col0 + width]
                nc.vector.tensor_tensor(t1, ps, cs, op=mybir.AluOpType.mult)
                nc.vector.tensor_tensor(t2[0:64, :], ps[64:128, :], sr[0:64, :],
                                        op=mybir.AluOpType.mult)
                nc.vector.tensor_tensor(t2[64:128, :], ps[0:64, :],
                                        sr[64:128, :], op=mybir.AluOpType.mult)
                nc.gpsimd.tensor_tensor(out_ap, t1, t2, op=mybir.AluOpType.add)

            # --- interleaved: xq staging, Q heads, x staging, K chunks ---
            # Q head h projects while x tile h stages; after head 4c+3 the
            # K chunk c (x tiles 4c..4c+3) projects.  PE never waits on DMA.
            with tc.tile_pool(name="qk", bufs=1) as qk_p:
                XTq = qk_p.tile([128, DT, TQ], BF16, tag="XTq", bufs=1)
                for ti in range(QT):
                    stage_x(xq, XTq[:, :, ti * 128:(ti + 1) * 128], ti, ti)

                kwT = []
                for h in range(H):
                    stage_x(x, XT[:, :, h * 128:(h + 1) * 128], h, h + QT)
                    wT = qk_p.tile([128, DT, 128], BF16, tag="qwT", bufs=2)
                    quant_tile(qw, h, out_T=wT[:])
                    ps = pps.tile([128, TQ], F32, tag="ps")
                    for dt in range(DT):
                        nc.tensor.matmul(ps, wT[:, dt, :], XTq[:, dt, :],
                                         start=(dt == 0), stop=(dt == DT - 1))
                    rope_evac(ps, cosq_sb, sinqr_sb, 0, TQ, QT_t[:, h, :])

                    if h == 0:
                        for hk in range(HK):
                            wTk = qk_p.tile([128, DT, 128], BF16, tag="kwT",
                                            bufs=4)
                            quant_tile(kw, hk, out_T=wTk[:])
                            kwT.append(wTk)

                    if h % 4 == 3:
                        tc4 = h // 4
                        for hk in range(HK):
                            ps = pps.tile([128, 512], F32, tag="ps")
                            for dt in range(DT):
                                nc.tensor.matmul(
                                    ps, kwT[hk][:, dt, :],
                                    XT[:, dt, tc4 * 512:(tc4 + 1) * 512],
                                    start=(dt == 0), stop=(dt == DT - 1))
                            rope_evac(ps, cosk_sb, sinkr_sb, tc4 * 512, 512,
                                      KT_t[:, hk, tc4 * 512:(tc4 + 1) * 512])

            # --- V projection + o-weight quant ---
            with tc.tile_pool(name="vq", bufs=1) as vq_p:
                vq = vq_p.tile([128, DT, HK * HD], BF16)
                for rv in range(HK * HD // 128):
                    quant_tile(vw, rv, out_T=vq[:, :, rv * 128:(rv + 1) * 128])
                for tch in range(NKT):
                    ps = pps.tile([128, HK * HD], F32, tag="ps")
                    for dt in range(DT):
                        nc.tensor.matmul(ps,
                                         XT[:, dt, tch * 128:(tch + 1) * 128],
                                         vq[:, dt, :],
                                         start=(dt == 0), stop=(dt == DT - 1))
                    nc.scalar.copy(V_t[:, tch, :], ps)
                    # o-weight quant engine work rides along with V proj
                    if tch % 2 == 0:
                        ro = tch // 2
                        quant_tile(ow, ro,
                                   out_rowmajor=wqd[ro * 128:(ro + 1) * 128, :],
                                   or_eng=nc.gpsimd)
                for ro in range(8, D // 128):
                    quant_tile(ow, ro,
                               out_rowmajor=wqd[ro * 128:(ro + 1) * 128, :],
                               or_eng=nc.gpsimd)

        # ============== phase 2: attention + output projection ==========
        with tc.tile_pool(name="att_res", bufs=1) as ares:

            OT_t = ares.tile([128, H, TQ], BF16)   # attn out^T [dv, h, q]
            owT = ares.tile([128, H, D], BF16)     # o-weights^T [dH, ht, dout]

            with tc.tile_pool(name="attn", bufs=2) as apool, \
                 tc.tile_pool(name="st_ps", bufs=2, space="PSUM") as stp, \
                 tc.tile_pool(name="sum_ps", bufs=1, space="PSUM") as sump, \
                 tc.tile_pool(name="o_ps", bufs=2, space="PSUM") as op:
                attention_heads(nc, tc, apool, stp, sump, op, KT_t, QT_t, V_t,
                                OT_t, owT, wqd, dmask_sb, ones16k)

            # --- output projection ---
            with tc.tile_pool(name="oproj", bufs=2) as opool, \
                 tc.tile_pool(name="op_ps", bufs=4, space="PSUM") as opp:
                for m in range(QT):
                    osb = opool.tile([128, D], F32, tag="osb")
                    for oc in range(4):
                        ps = opp.tile([128, 512], F32, tag="ps")
                        for ht in range(H):
                            nc.tensor.matmul(ps,
                                             OT_t[:, ht, m * 128:(m + 1) * 128],
                                             owT[:, ht, oc * 512:(oc + 1) * 512],
                                             start=(ht == 0), stop=(ht == H - 1))
                        if oc % 2 == 0:
                            nc.vector.tensor_copy(
                                osb[:, oc * 512:(oc + 1) * 512], ps)
                        else:
                            nc.scalar.copy(osb[:, oc * 512:(oc + 1) * 512], ps)
                    nc.sync.dma_start(out[m * 128:(m + 1) * 128, :], osb)


def attention_heads(nc, tc, apool, stp, sump, op, KT_t, QT_t, V_t, OT_t, owT,
                    wqd, dmask_sb, ones16k):
            for h in range(H):
                hk = h // 4
                ps_o = op.tile([128, TQ], F32, tag="ps_o")
                ps_sum = sump.tile([128, TQ], F32, tag="ps_sum")
                for p in range(NPAIR):
                    kt0, kt1 = 2 * p, 2 * p + 1
                    q0, q1 = 32 * kt0, 32 * kt1
                    ps_st = stp.tile([128, 2 * TQ], F32, tag="ps_st")
                    nc.tensor.matmul(ps_st[:, q0:TQ],
                                     KT_t[:, hk, kt0 * 128:(kt0 + 1) * 128],
                                     QT_t[:, h, q0:], start=True, stop=True)
                    nc.tensor.matmul(ps_st[:, TQ + q1:2 * TQ],
                                     KT_t[:, hk, kt1 * 128:(kt1 + 1) * 128],
                                     QT_t[:, h, q1:], start=True, stop=True)
                    pt = apool.tile([128, 2 * TQ], BF16, tag="pt", bufs=4)
                    # one exp op over both halves, strided to skip the gap
                    nc.scalar.activation(
                        pt.rearrange("p (k q) -> p k q", k=2)[:, :, q0:],
                        ps_st.rearrange("p (k q) -> p k q", k=2)[:, :, q0:],
                        mybir.ActivationFunctionType.Exp)
                    # diagonal strip masks (multiplicative 0/1)
                    nc.gpsimd.tensor_tensor(pt[:, q0:q0 + 32], pt[:, q0:q0 + 32],
                                            dmask_sb, op=mybir.AluOpType.mult)
                    nc.gpsimd.tensor_tensor(pt[:, TQ + q1:TQ + q1 + 32],
                                            pt[:, TQ + q1:TQ + q1 + 32],
                                            dmask_sb, op=mybir.AluOpType.mult)
                    # denominator + attn@V accumulation
                    nc.tensor.matmul(ps_sum[:, q0:], ones16k, pt[:, q0:TQ],
                                     start=(p == 0), stop=False)
                    nc.tensor.matmul(ps_sum[:, q1:], ones16k,
                                     pt[:, TQ + q1:2 * TQ],
                                     start=False, stop=(p == NPAIR - 1))
                    nc.tensor.matmul(ps_o[:, q0:],
                                     V_t[:, kt0, hk * HD:(hk + 1) * HD],
                                     pt[:, q0:TQ], start=(p == 0), stop=False)
                    nc.tensor.matmul(ps_o[:, q1:],
                                     V_t[:, kt1, hk * HD:(hk + 1) * HD],
                                     pt[:, TQ + q1:2 * TQ],
                                     start=False, stop=(p == NPAIR - 1))
                # sums arrive broadcast on all 128 partitions: fast reciprocal
                RQb = apool.tile([128, TQ], F32, tag="RQb", bufs=2)
                nc.vector.reciprocal_approx_fast(RQb, ps_sum)
                nc.vector.tensor_tensor(OT_t[:, h, :], ps_o, RQb,
                                        op=mybir.AluOpType.mult)
                # interleave owT transposes with attention
                if h >= 8:
                    for ht in (2 * (h - 8), 2 * (h - 8) + 1):
                        nc.sync.dma_start_transpose(
                            owT[:, ht, :], wqd[:, ht * 128:(ht + 1) * 128])


# ---------------------------------------------------------------------------
# host side
# ---------------------------------------------------------------------------
_CACHE = {}


def _tables():
    inv = 1.0 / (THETA ** (np.arange(0, HD, 2, dtype=np.float64) / HD))
    t = np.arange(T, dtype=np.float64)
    fr = np.outer(t, inv)                      # [T, 64]
    emb = np.concatenate([fr, fr], axis=1)     # [T, 128]
    cosT = np.cos(emb).T                       # [128, T] float64
    sinT = np.sin(emb).T
    sinr = np.empty_like(sinT)
    sinr[0:64] = -sinT[0:64]
    sinr[64:128] = sinT[64:128]
    return cosT, sinT, sinr


def make_in_maps(hidden, q_w, k_w, v_w, o_w):
    cosT, sinT, sinr = _tables()
    f16 = np.float16
    in_maps = []
    for c in range(NC):
        b, i = c // 4, c % 4
        xb_ = np.ascontiguousarray(hidden[b])
        xq_ = np.ascontiguousarray(hidden[b][i::4, :])
        cq = np.ascontiguousarray(cosT[:, i::4] * ALPHA_Q).astype(f16)
        sq = np.ascontiguousarray(sinr[:, i::4] * ALPHA_Q).astype(f16)
        # dmask[r, c] = 1 iff key-local r <= 4c + i (diagonal 128x32 strip)
        r = np.arange(128)[:, None]
        cc = np.arange(32)[None, :]
        dm = (r <= 4 * cc + i).astype(ml_dtypes.bfloat16)
        in_maps.append({
            "x": xb_, "xq": xq_, "qw": q_w, "kw": k_w, "vw": v_w, "ow": o_w,
            "cosk": np.ascontiguousarray(cosT * ALPHA_K).astype(f16),
            "sinkr": np.ascontiguousarray(sinr * ALPHA_K).astype(f16),
            "cosq": cq, "sinqr": sq, "dmask": dm,
        })
    return in_maps


def kernel(hidden, q_w, k_w, v_w, o_w):
    hidden = np.asarray(hidden, dtype=np.float32)
    q_w = np.ascontiguousarray(np.asarray(q_w, dtype=np.float32))
    k_w = np.ascontiguousarray(np.asarray(k_w, dtype=np.float32))
    v_w = np.ascontiguousarray(np.asarray(v_w, dtype=np.float32))
    o_w = np.ascontiguousarray(np.asarray(o_w, dtype=np.float32))

    if "nc" not in _CACHE:
        _CACHE["nc"] = build_program()
    nc = _CACHE["nc"]

    in_maps = make_in_maps(hidden, q_w, k_w, v_w, o_w)
    from concourse.bass_utils import run_bass_kernel_spmd
    res = run_bass_kernel_spmd(nc, in_maps, core_ids=list(range(NC)))
    out = np.empty((B, T, D), dtype=np.float32)
    for c in range(NC):
        b, i = c // 4, c % 4
        out[b, i::4, :] = res.results[c]["out"]
    return out


if __name__ == "__main__":
    print("building program...")
    nc = build_program()
    print("BUILD OK")


# revision 37
# speedup vs baseline: 1.2870x; 1.1527x over previous
"""Trainium2 Bass kernel for GroupedQueryAttention with 1-bit quantized linears.

Sharding: 8 cores = 2 batches x 4 token-interleaved groups.
Core c handles batch b=c//4 and tokens t with t%4 == i (i=c%4), i.e. 512
query tokens per core.  Every core computes full K/V for its batch
(replicated), all 16 heads for its own 512 queries, and the full O
projection for those rows.  Host gathers by re-interleaving rows.

v2 restructure vs baseline:
 - no DRAM bf16 round-trip for x: f32 tiles load straight to SBUF,
   engines cast to bf16, SBUF->SBUF DMA transpose into XT.
 - weight quantization reads the f32 staging tile directly (reduce DVE,
   sign ACT, scale-mult gpsimd/DVE).
 - exp over PAIRED score tiles ([128,1024] PSUM spans 2 banks) to halve
   ACT per-op overhead.
 - softmax reciprocal broadcast via K=1 matmul instead of DRAM trip.
 - o-weight quant overlaps K/V projections; owT transpose overlaps
   attention.

Program is identical across cores; all per-core variation is input data.
"""

import sys

sys.path.insert(0, "/opt/trn_rl_repo")

import numpy as np
import ml_dtypes

import concourse.bacc as bacc
import concourse.bass as bass
import concourse.mybir as mybir
import concourse.tile as tile

F32 = mybir.dt.float32
F16 = mybir.dt.float16
BF16 = mybir.dt.bfloat16

B, T, D = 2, 2048, 2048
H, HK, HD = 16, 4, 128
G = 128
THETA = 1000000.0
NC = 8
TQ = T // 4          # 512 query tokens per core
QT = TQ // 128       # 4 query tiles
DT = D // 128        # 16 din tiles
NKT = T // 128       # 16 key tiles
NPAIR = NKT // 2     # 8 key-tile pairs

ALPHA_K = 1.0 / G
ALPHA_Q = (HD ** -0.5) / G


def _bcast(ap_small, like_ap):
    a, b = bass.broadcast_tensor_aps(like_ap, ap_small)
    return b


def build_program():
    nc = bacc.Bacc("TRN2", target_bir_lowering=False, debug=False, num_devices=NC)

    x = nc.dram_tensor("x", [T, D], F32, kind="ExternalInput").ap()
    xq = nc.dram_tensor("xq", [TQ, D], F32, kind="ExternalInput").ap()
    qw = nc.dram_tensor("qw", [H * HD, D], F32, kind="ExternalInput").ap()
    kw = nc.dram_tensor("kw", [HK * HD, D], F32, kind="ExternalInput").ap()
    vw = nc.dram_tensor("vw", [HK * HD, D], F32, kind="ExternalInput").ap()
    ow = nc.dram_tensor("ow", [D, H * HD], F32, kind="ExternalInput").ap()
    cosk = nc.dram_tensor("cosk", [HD, T], F16, kind="ExternalInput").ap()
    sinkr = nc.dram_tensor("sinkr", [HD, T], F16, kind="ExternalInput").ap()
    cosq = nc.dram_tensor("cosq", [HD, TQ], F16, kind="ExternalInput").ap()
    sinqr = nc.dram_tensor("sinqr", [HD, TQ], F16, kind="ExternalInput").ap()
    dmask = nc.dram_tensor("dmask", [128, 32], BF16, kind="ExternalInput").ap()
    out = nc.dram_tensor("out", [TQ, D], F32, kind="ExternalOutput").ap()

    with tile.TileContext(nc) as tc:
        build_tile_kernel(nc, tc, x, xq, qw, kw, vw, ow, cosk, sinkr, cosq,
                          sinqr, dmask, out)
    nc.compile()
    return nc


def build_tile_kernel(nc, tc, x, xq, qw, kw, vw, ow, cosk, sinkr, cosq, sinqr,
                      dmask, out):
    from contextlib import ExitStack

    ctx = ExitStack()
    with ctx:
        # ------- long-lived pools --------
        dram = ctx.enter_context(tc.tile_pool(name="dram", bufs=1, space="DRAM"))
        const = ctx.enter_context(tc.tile_pool(name="const", bufs=1))
        resid = ctx.enter_context(tc.tile_pool(name="resid", bufs=1))

        wqd = dram.tile([D, H * HD], BF16)      # quantized o-weights (row major)

        dmask_sb = const.tile([128, 32], BF16)
        nc.sync.dma_start(dmask_sb, dmask)
        # [128,128] of G*G: sum-matmul output lands broadcast on all 128
        # partitions, so the reciprocal runs on 128 lanes (no [1,512] crawl)
        ones16k = const.tile([128, 128], BF16)
        nc.gpsimd.memset(ones16k, float(G * G))

        cosk_sb = const.tile([128, T], F16)
        sinkr_sb = const.tile([128, T], F16)
        cosq_sb = const.tile([128, TQ], F16)
        sinqr_sb = const.tile([128, TQ], F16)
        nc.sync.dma_start(cosk_sb, cosk)
        nc.sync.dma_start(sinkr_sb, sinkr)
        nc.sync.dma_start(cosq_sb, cosq)
        nc.sync.dma_start(sinqr_sb, sinqr)

        # residents alive through attention (48 KB/partition)
        QT_t = resid.tile([128, H, TQ], BF16)    # roped q^T  [d, h, t]
        KT_t = resid.tile([128, HK, T], BF16)    # roped k^T  [d, hk, t]
        V_t = resid.tile([128, NKT, HK * HD], BF16)  # v [t, kv-dim]

        # ============== phase 1: staging + projections ==================
        with tc.tile_pool(name="xtp", bufs=1) as pxt, \
             tc.tile_pool(name="wst", bufs=3) as wst_p, \
             tc.tile_pool(name="wqt", bufs=3) as wqt_p, \
             tc.tile_pool(name="ssum", bufs=2) as ssum_p, \
             tc.tile_pool(name="rtmp", bufs=2) as rtmp, \
             tc.tile_pool(name="proj_ps", bufs=4, space="PSUM") as pps:

            XT = pxt.tile([128, DT, T], BF16)     # x^T (din-major), 64 KB/p
            xd = dram.tile([T, D], BF16)          # bf16 x staging in DRAM
            xqd = dram.tile([TQ, D], BF16)

            U16 = mybir.dt.uint16
            U32 = mybir.dt.uint32

            def quant_tile(w_ap, row_tile, out_T=None, out_rowmajor=None,
                           or_eng=None):
                """Load 128 rows of w as bf16, 1-bit quantize (x G scale).

                Copysign in u32 bit-space over bf16 pairs:
                wq = (w & 0x80008000) | ((scale<<16)|scale).
                Single-engine (DVE) chain after the gpsimd cast-load.
                """
                wst = wst_p.tile([128, D], BF16, tag="wst")
                nc.gpsimd.dma_start(
                    wst, w_ap[row_tile * 128:(row_tile + 1) * 128, :])
                ssum = ssum_p.tile([128, DT], F32, tag="ssum")
                nc.vector.tensor_reduce(
                    ssum, wst.rearrange("p (g c) -> p g c", c=G),
                    axis=mybir.AxisListType.X, op=mybir.AluOpType.add,
                    apply_absolute_value=True)
                # duplicated bf16 scales -> u32 (s,s) pairs via two copies
                ssb2 = ssum_p.tile([128, 2 * DT], BF16, tag="ssb2")
                sd = ssb2.rearrange("p (g two) -> p g two", two=2)
                nc.vector.tensor_copy(sd[:, :, 0:1],
                                      ssum.rearrange("p (g o) -> p g o", o=1))
                nc.vector.tensor_copy(sd[:, :, 1:2],
                                      ssum.rearrange("p (g o) -> p g o", o=1))
                spair = ssb2[:].bitcast(U32)
                wqt = wqt_p.tile([128, D], BF16, tag="wqt")
                nc.vector.tensor_scalar(
                    wqt[:].bitcast(U32), wst[:].bitcast(U32), 0x80008000, None,
                    op0=mybir.AluOpType.bitwise_and)
                sv = spair.rearrange("p (g o) -> p g o", o=1)
                gv = wqt[:].bitcast(U32).rearrange("p (g c) -> p g c", c=G // 2)
                nc.vector.tensor_tensor(
                    gv, gv, _bcast(sv, gv), op=mybir.AluOpType.bitwise_or)
                if out_T is not None:
                    nc.sync.dma_start_transpose(out_T, wqt[:])
                if out_rowmajor is not None:
                    nc.sync.dma_start(out_rowmajor, wqt[:])

            def rope_evac(ps, cos_sb, sinr_sb, col0, width, out_ap):
                """out = ps*cos + rot(ps)*sinr  (cast bf16).

                ACT evacuates PSUM first (it has the fast PSUM port); DVE
                multiplies run on SBUF at full rate.
                """
                pse = rtmp.tile([128, width], F32, tag="pse", bufs=2)
                nc.scalar.copy(pse, ps)
                t1 = rtmp.tile([128, width], F32, tag="t1", bufs=2)
                t2 = rtmp.tile([128, width], F32, tag="t2", bufs=2)
                cs = cos_sb[:, col0:col0 + width]
                sr = sinr_sb[:, col0:col0 + width]
                # sinr tables arrive pre-rolled by 64 partitions so that the
                # rotate-half reads share a base partition with the table
                nc.vector.tensor_tensor(t1, pse, cs, op=mybir.AluOpType.mult)
                nc.vector.tensor_tensor(t2[0:64, :], pse[64:128, :],
                                        sr[64:128, :], op=mybir.AluOpType.mult)
                nc.vector.tensor_tensor(t2[64:128, :], pse[0:64, :],
                                        sr[0:64, :], op=mybir.AluOpType.mult)
                nc.gpsimd.tensor_tensor(out_ap, t1, t2, op=mybir.AluOpType.add)

            # --- interleaved: xq staging, Q heads, x staging, K chunks ---
            # x casts to bf16 in DRAM (gpsimd cast-DMA, no engine work), then
            # DRAM->SBUF transposes per 512-token chunk; K chunk c projects
            # after head 4c+3 so PE never waits on staging.
            with tc.tile_pool(name="qk", bufs=1) as qk_p:
                XTq = qk_p.tile([128, DT, TQ], BF16, tag="XTq", bufs=1)
                nc.gpsimd.dma_start(xqd[:], xq)
                nc.sync.dma_start_transpose(XTq[:], xqd[:])

                kwT = []
                for h in range(H):
                    if h % 4 == 0:
                        c = h // 4
                        nc.gpsimd.dma_start(xd[c * 512:(c + 1) * 512, :],
                                            x[c * 512:(c + 1) * 512, :])
                        nc.sync.dma_start_transpose(
                            XT[:, :, c * 512:(c + 1) * 512],
                            xd[c * 512:(c + 1) * 512, :])
                    wT = qk_p.tile([128, DT, 128], BF16, tag="qwT", bufs=3)
                    quant_tile(qw, h, out_T=wT[:])
                    ps = pps.tile([128, TQ], F32, tag="ps")
                    for dt in range(DT):
                        nc.tensor.matmul(ps, wT[:, dt, :], XTq[:, dt, :],
                                         start=(dt == 0), stop=(dt == DT - 1))
                    rope_evac(ps, cosq_sb, sinqr_sb, 0, TQ, QT_t[:, h, :])

                    if h == 0:
                        for hk in range(HK):
                            wTk = qk_p.tile([128, DT, 128], BF16, tag="kwT",
                                            bufs=4)
                            quant_tile(kw, hk, out_T=wTk[:])
                            kwT.append(wTk)

                    if h % 4 == 3:
                        tc4 = h // 4
                        for hk in range(HK):
                            ps = pps.tile([128, 512], F32, tag="ps")
                            for dt in range(DT):
                                nc.tensor.matmul(
                                    ps, kwT[hk][:, dt, :],
                                    XT[:, dt, tc4 * 512:(tc4 + 1) * 512],
                                    start=(dt == 0), stop=(dt == DT - 1))
                            rope_evac(ps, cosk_sb, sinkr_sb, tc4 * 512, 512,
                                      KT_t[:, hk, tc4 * 512:(tc4 + 1) * 512])

            # --- V projection + o-weight quant ---
            with tc.tile_pool(name="vq", bufs=1) as vq_p:
                vq = vq_p.tile([128, DT, HK * HD], BF16)
                for rv in range(HK * HD // 128):
                    quant_tile(vw, rv, out_T=vq[:, :, rv * 128:(rv + 1) * 128])
                for tch in range(NKT):
                    ps = pps.tile([128, HK * HD], F32, tag="ps")
                    for dt in range(DT):
                        nc.tensor.matmul(ps,
                                         XT[:, dt, tch * 128:(tch + 1) * 128],
                                         vq[:, dt, :],
                                         start=(dt == 0), stop=(dt == DT - 1))
                    nc.scalar.copy(V_t[:, tch, :], ps)
                    # o-weight quant engine work rides along with V proj
                    quant_tile(ow, tch,
                               out_rowmajor=wqd[tch * 128:(tch + 1) * 128, :])

        # ============== phase 2: attention + output projection ==========
        with tc.tile_pool(name="att_res", bufs=1) as ares:

            OT_t = ares.tile([128, H, TQ], BF16)   # attn out^T [dv, h, q]
            owT = ares.tile([128, H, D], BF16)     # o-weights^T [dH, ht, dout]

            with tc.tile_pool(name="attn", bufs=2) as apool, \
                 tc.tile_pool(name="st_ps", bufs=2, space="PSUM") as stp, \
                 tc.tile_pool(name="sum_ps", bufs=1, space="PSUM") as sump, \
                 tc.tile_pool(name="o_ps", bufs=2, space="PSUM") as op:
                attention_heads(nc, tc, apool, stp, sump, op, KT_t, QT_t, V_t,
                                OT_t, owT, wqd, dmask_sb, ones16k)

            # --- output projection ---
            with tc.tile_pool(name="oproj", bufs=2) as opool, \
                 tc.tile_pool(name="op_ps", bufs=4, space="PSUM") as opp:
                for m in range(QT):
                    osb = opool.tile([128, D], F32, tag="osb")
                    for oc in range(4):
                        ps = opp.tile([128, 512], F32, tag="ps")
                        for ht in range(H):
                            nc.tensor.matmul(ps,
                                             OT_t[:, ht, m * 128:(m + 1) * 128],
                                             owT[:, ht, oc * 512:(oc + 1) * 512],
                                             start=(ht == 0), stop=(ht == H - 1))
                        if oc % 2 == 0:
                            nc.vector.tensor_copy(
                                osb[:, oc * 512:(oc + 1) * 512], ps)
                        else:
                            nc.scalar.copy(osb[:, oc * 512:(oc + 1) * 512], ps)
                    nc.sync.dma_start(out[m * 128:(m + 1) * 128, :], osb)


def attention_heads(nc, tc, apool, stp, sump, op, KT_t, QT_t, V_t, OT_t, owT,
                    wqd, dmask_sb, ones16k):
            for h in range(H):
                hk = h // 4
                ps_o = op.tile([128, TQ], F32, tag="ps_o")
                ps_sum = sump.tile([128, TQ], F32, tag="ps_sum")
                for p in range(NPAIR):
                    kt0, kt1 = 2 * p, 2 * p + 1
                    q0, q1 = 32 * kt0, 32 * kt1
                    ps_st = stp.tile([128, 2 * TQ], F32, tag="ps_st")
                    nc.tensor.matmul(ps_st[:, q0:TQ],
                                     KT_t[:, hk, kt0 * 128:(kt0 + 1) * 128],
                                     QT_t[:, h, q0:], start=True, stop=True)
                    nc.tensor.matmul(ps_st[:, TQ + q1:2 * TQ],
                                     KT_t[:, hk, kt1 * 128:(kt1 + 1) * 128],
                                     QT_t[:, h, q1:], start=True, stop=True)
                    pt = apool.tile([128, 2 * TQ], BF16, tag="pt", bufs=4)
                    # one exp op over both halves, strided to skip the gap
                    nc.scalar.activation(
                        pt.rearrange("p (k q) -> p k q", k=2)[:, :, q0:],
                        ps_st.rearrange("p (k q) -> p k q", k=2)[:, :, q0:],
                        mybir.ActivationFunctionType.Exp)
                    # diagonal strip masks (multiplicative 0/1)
                    nc.gpsimd.tensor_tensor(pt[:, q0:q0 + 32], pt[:, q0:q0 + 32],
                                            dmask_sb, op=mybir.AluOpType.mult)
                    nc.gpsimd.tensor_tensor(pt[:, TQ + q1:TQ + q1 + 32],
                                            pt[:, TQ + q1:TQ + q1 + 32],
                                            dmask_sb, op=mybir.AluOpType.mult)
                    # denominator + attn@V accumulation
                    nc.tensor.matmul(ps_sum[:, q0:], ones16k, pt[:, q0:TQ],
                                     start=(p == 0), stop=False)
                    nc.tensor.matmul(ps_sum[:, q1:], ones16k,
                                     pt[:, TQ + q1:2 * TQ],
                                     start=False, stop=(p == NPAIR - 1))
                    nc.tensor.matmul(ps_o[:, q0:],
                                     V_t[:, kt0, hk * HD:(hk + 1) * HD],
                                     pt[:, q0:TQ], start=(p == 0), stop=False)
                    nc.tensor.matmul(ps_o[:, q1:],
                                     V_t[:, kt1, hk * HD:(hk + 1) * HD],
                                     pt[:, TQ + q1:2 * TQ],
                                     start=False, stop=(p == NPAIR - 1))
                # sums arrive broadcast on all 128 partitions: fast reciprocal
                RQb = apool.tile([128, TQ], F32, tag="RQb", bufs=2)
                nc.vector.reciprocal_approx_fast(RQb, ps_sum)
                nc.vector.tensor_tensor(OT_t[:, h, :], ps_o, RQb,
                                        op=mybir.AluOpType.mult)
                # interleave owT transposes with attention
                if h >= 8:
                    for ht in (2 * (h - 8), 2 * (h - 8) + 1):
                        nc.sync.dma_start_transpose(
                            owT[:, ht, :], wqd[:, ht * 128:(ht + 1) * 128])


# ---------------------------------------------------------------------------
# host side
# ---------------------------------------------------------------------------
_CACHE = {}


def _tables():
    inv = 1.0 / (THETA ** (np.arange(0, HD, 2, dtype=np.float64) / HD))
    t = np.arange(T, dtype=np.float64)
    fr = np.outer(t, inv)                      # [T, 64]
    emb = np.concatenate([fr, fr], axis=1)     # [T, 128]
    cosT = np.cos(emb).T                       # [128, T] float64
    sinT = np.sin(emb).T
    sinr = np.empty_like(sinT)
    sinr[0:64] = -sinT[0:64]
    sinr[64:128] = sinT[64:128]
    # rolled by 64 partitions: kernel reads srs[64:128] for out[0:64] etc.
    sinr = np.roll(sinr, 64, axis=0)
    return cosT, sinT, sinr


def make_in_maps(hidden, q_w, k_w, v_w, o_w):
    cosT, sinT, sinr = _tables()
    f16 = np.float16
    in_maps = []
    for c in range(NC):
        b, i = c // 4, c % 4
        xb_ = np.ascontiguousarray(hidden[b])
        xq_ = np.ascontiguousarray(hidden[b][i::4, :])
        cq = np.ascontiguousarray(cosT[:, i::4] * ALPHA_Q).astype(f16)
        sq = np.ascontiguousarray(sinr[:, i::4] * ALPHA_Q).astype(f16)
        # dmask[r, c] = 1 iff key-local r <= 4c + i (diagonal 128x32 strip)
        r = np.arange(128)[:, None]
        cc = np.arange(32)[None, :]
        dm = (r <= 4 * cc + i).astype(ml_dtypes.bfloat16)
        in_maps.append({
            "x": xb_, "xq": xq_, "qw": q_w, "kw": k_w, "vw": v_w, "ow": o_w,
            "cosk": np.ascontiguousarray(cosT * ALPHA_K).astype(f16),
            "sinkr": np.ascontiguousarray(sinr * ALPHA_K).astype(f16),
            "cosq": cq, "sinqr": sq, "dmask": dm,
        })
    return in_maps


def kernel(hidden, q_w, k_w, v_w, o_w):
    hidden = np.asarray(hidden, dtype=np.float32)
    q_w = np.ascontiguousarray(np.asarray(q_w, dtype=np.float32))
    k_w = np.ascontiguousarray(np.asarray(k_w, dtype=np.float32))
    v_w = np.ascontiguousarray(np.asarray(v_w, dtype=np.float32))
    o_w = np.ascontiguousarray(np.asarray(o_w, dtype=np.float32))

    if "nc" not in _CACHE:
        _CACHE["nc"] = build_program()
    nc = _CACHE["nc"]

    in_maps = make_in_maps(hidden, q_w, k_w, v_w, o_w)
    from concourse.bass_utils import run_bass_kernel_spmd
    res = run_bass_kernel_spmd(nc, in_maps, core_ids=list(range(NC)))
    out = np.empty((B, T, D), dtype=np.float32)
    for c in range(NC):
        b, i = c // 4, c % 4
        out[b, i::4, :] = res.results[c]["out"]
    return out


if __name__ == "__main__":
    print("building program...")
    nc = build_program()
    print("BUILD OK")
